# revision 1
# baseline (speedup 1.0000x reference)
"""Trainium2 Bass kernel for nn_Net_21586505630402 (2-layer GraphSAGE mean-aggr GNN).

Strategy (8 NeuronCores, SPMD single NEFF):
  - Destination nodes are packed into 784 bins of <=128 nodes, balanced by edge
    count (greedy, node-capacity 128). 98 bins per core.
  - Per bin: edge messages x[src] are fetched with indirect DMA (rows cast
    f32->bf16 on the fly), segment-summed into PSUM via a one-hot matmul
    (one-hot built on-device with is_equal against an iota row), divided by
    degree, then lin_l/lin_r dense matmuls (fp32) with PE transposes.
  - Layer-1 output x1 is written fp32 (local) + bf16 (AllGather across the 8
    cores) so layer 2 can gather any row. Self rows use the fp32 copy.
  - Gather index lists are sorted ascending per bin for HBM locality.
Host side does only integer index preprocessing (sharding/permutation) and
final unpermute/assembly.
"""

import heapq
import numpy as np
import ml_dtypes

import concourse.bass as bass
import concourse.bacc as bacc
import concourse.tile as tile
import concourse.mybir as mybir
from concourse.bass_utils import run_bass_kernel_spmd
from concourse.masks import make_identity

NC = 8
P = 128
D = 256
N_NODES = 100000
N_EMB = 100001
E_TOTAL = 1600000
BINS_PER_CORE = 98
NBINS = NC * BINS_PER_CORE          # 784
NODE_SLOTS = BINS_PER_CORE * P      # 12544 per core
F32 = mybir.dt.float32
BF16 = mybir.dt.bfloat16
I32 = mybir.dt.int32


# ---------------------------------------------------------------- host side --

def _partition_nodes(deg):
    """Greedy balanced binning: nodes -> (bin, slot); <=128 nodes/bin,
    minimize max edge count per bin. Deterministic."""
    n = deg.shape[0]
    order = np.argsort(-deg, kind="stable")
    node2bin = np.empty(n, dtype=np.int32)
    node2slot = np.empty(n, dtype=np.int32)
    bin_edges = np.zeros(NBINS, dtype=np.int64)
    bin_nodes = np.zeros(NBINS, dtype=np.int32)
    heap = [(0, b) for b in range(NBINS)]
    heapq.heapify(heap)
    spill = []
    for node in order:
        while True:
            s, b = heapq.heappop(heap)
            if bin_nodes[b] < P:
                break
            spill.append((s, b))  # full bin, drop from rotation
        node2bin[node] = b
        node2slot[node] = bin_nodes[b]
        bin_nodes[b] += 1
        bin_edges[b] += int(deg[node])
        if bin_nodes[b] < P:
            heapq.heappush(heap, (int(bin_edges[b]), b))
    return node2bin, node2slot, bin_edges


def _slot_arrays(bins_e, key, dslot_e, T, core_of_bin_local):
    """Given per-edge (bin, sortkey, dstslot), build per-core
    [128, BINS_PER_CORE*T] gather-index and dstf arrays."""
    order = np.lexsort((key, bins_e))
    b_sorted = bins_e[order]
    k_sorted = key[order]
    d_sorted = dslot_e[order]
    counts = np.bincount(b_sorted, minlength=NBINS)
    starts = np.concatenate([[0], np.cumsum(counts)[:-1]])
    pos = np.arange(b_sorted.size) - starts[b_sorted]   # rank within bin
    t_idx = (pos // P).astype(np.int64)
    row = (pos % P).astype(np.int64)
    assert t_idx.max() < T
    core = b_sorted // BINS_PER_CORE
    binloc = b_sorted % BINS_PER_CORE
    col = binloc * T + t_idx
    gidx = np.zeros((NC, P, BINS_PER_CORE * T), dtype=np.int32)
    dstf = np.full((NC, P, BINS_PER_CORE * T), -1.0, dtype=np.float32)
    gidx[core, row, col] = k_sorted.astype(np.int32)
    dstf[core, row, col] = d_sorted.astype(np.float32)
    return gidx, dstf


def prepare_host(inputs):
    emb = np.ascontiguousarray(np.asarray(inputs["emb"], dtype=np.float32))
    W1l = np.asarray(inputs["W1l"], dtype=np.float32)
    b1l = np.asarray(inputs["b1l"], dtype=np.float32).reshape(1, D)
    W1r = np.asarray(inputs["W1r"], dtype=np.float32)
    W2l = np.asarray(inputs["W2l"], dtype=np.float32)
    b2l = np.asarray(inputs["b2l"], dtype=np.float32).reshape(1, D)
    W2r = np.asarray(inputs["W2r"], dtype=np.float32)
    x_idx = np.asarray(inputs["x_idx"]).astype(np.int64)
    edge = np.asarray(inputs["edge_index"]).astype(np.int64)
    src, dst = edge[0], edge[1]

    deg = np.bincount(dst, minlength=N_NODES).astype(np.int64)
    node2bin, node2slot, bin_edges = _partition_nodes(deg)
    T = max(16, int(np.ceil(bin_edges.max() / P)))

    bins_e = node2bin[dst]
    dslot_e = node2slot[dst]
    # layer-1 gather: emb row = x_idx[src]
    g1_e = x_idx[src]
    # layer-2 gather: allgathered-x1 row of node src
    agrow = (node2bin.astype(np.int64) // BINS_PER_CORE) * NODE_SLOTS \
        + (node2bin.astype(np.int64) % BINS_PER_CORE) * P + node2slot
    g2_e = agrow[src]

    g1, df1 = _slot_arrays(bins_e, g1_e, dslot_e, T, None)
    g2, df2 = _slot_arrays(bins_e, g2_e, dslot_e, T, None)
    # pads: g1 already 0 -> make them row N_NODES (exists, harmless);
    # pad rows only matter for DMA validity, their one-hot col is 0.
    g1[df1 < 0] = N_NODES
    g2[df2 < 0] = 0

    # self rows / counts, [128, BINS_PER_CORE] per core
    selfi = np.full((NC, P, BINS_PER_CORE), N_NODES, dtype=np.int32)
    cnt = np.zeros((NC, P, BINS_PER_CORE), dtype=np.float32)
    core_n = node2bin // BINS_PER_CORE
    binloc_n = node2bin % BINS_PER_CORE
    selfi[core_n, node2slot, binloc_n] = x_idx.astype(np.int32)
    cnt[core_n, node2slot, binloc_n] = deg.astype(np.float32)

    iota = np.broadcast_to(np.arange(P, dtype=np.float32), (P, P)).copy()

    in_maps = []
    for c in range(NC):
        in_maps.append({
            "emb": emb, "iota": iota,
            "W1l": W1l, "W1r": W1r, "W2l": W2l, "W2r": W2r,
            "b1": b1l, "b2": b2l,
            "g1": g1[c], "df1": df1[c], "g2": g2[c], "df2": df2[c],
            "selfi": selfi[c], "cnt": cnt[c],
        })

    info = {
        "node2bin": node2bin, "node2slot": node2slot, "T": T,
        "drug": np.asarray(inputs["drugNodes"]).astype(np.int64),
        "se": np.asarray(inputs["seNodes"]).astype(np.int64),
    }
    return in_maps, info


def assemble(results, info):
    node2bin, node2slot = info["node2bin"], info["node2slot"]
    x = np.empty((N_NODES, D), dtype=np.float32)
    core_n = node2bin // BINS_PER_CORE
    row_n = (node2bin % BINS_PER_CORE) * P + node2slot
    for c in range(NC):
        m = core_n == c
        x[np.nonzero(m)[0]] = results[c]["x2"][row_n[m]]
    drug = x[info["drug"]]
    se = x[info["se"]]
    return (drug, se, x)


# -------------------------------------------------------------- device side --

def build_program(T):
    nc = bacc.Bacc(None, target_bir_lowering=False, num_devices=NC)
    NTC = BINS_PER_CORE * T

    emb = nc.dram_tensor("emb", [N_EMB, D], F32, kind="ExternalInput")
    iota_d = nc.dram_tensor("iota", [P, P], F32, kind="ExternalInput")
    Wd = {w: nc.dram_tensor(w, [D, D], F32, kind="ExternalInput")
          for w in ("W1l", "W1r", "W2l", "W2r")}
    bd = {b: nc.dram_tensor(b, [1, D], F32, kind="ExternalInput")
          for b in ("b1", "b2")}
    g1_d = nc.dram_tensor("g1", [P, NTC], I32, kind="ExternalInput")
    df1_d = nc.dram_tensor("df1", [P, NTC], F32, kind="ExternalInput")
    g2_d = nc.dram_tensor("g2", [P, NTC], I32, kind="ExternalInput")
    df2_d = nc.dram_tensor("df2", [P, NTC], F32, kind="ExternalInput")
    selfi_d = nc.dram_tensor("selfi", [P, BINS_PER_CORE], I32, kind="ExternalInput")
    cnt_d = nc.dram_tensor("cnt", [P, BINS_PER_CORE], F32, kind="ExternalInput")
    x2_d = nc.dram_tensor("x2", [NODE_SLOTS, D], F32, kind="ExternalOutput")

    with tile.TileContext(nc, num_cores=NC) as tc:
        with (
            tc.tile_pool(name="const", bufs=1) as const,
            tc.tile_pool(name="gp", bufs=24) as gp,
            tc.tile_pool(name="ohp", bufs=8) as ohp,
            tc.tile_pool(name="selfp", bufs=3) as selfp,
            tc.tile_pool(name="work", bufs=3) as work,
            tc.tile_pool(name="outp", bufs=3) as outp,
            tc.tile_pool(name="psA", bufs=2, space="PSUM") as psA,
            tc.tile_pool(name="psT", bufs=2, space="PSUM") as psT,
            tc.tile_pool(name="psO", bufs=2, space="PSUM") as psO,
            tc.tile_pool(name="dram", bufs=1, space="DRAM") as dram,
        ):
            # ---- constants
            ident = const.tile([P, P], F32)
            make_identity(nc, ident[:])
            iota_sb = const.tile([P, P], F32)
            nc.sync.dma_start(iota_sb[:], iota_d[:])
            ones_sb = const.tile([1, P], F32)
            nc.vector.memset(ones_sb[:], 1.0)
            W_sb = {}
            for w in ("W1l", "W1r", "W2l", "W2r"):
                W_sb[w] = const.tile([P, 2 * D], F32, name=f"W_{w}")
                nc.sync.dma_start(W_sb[w][:, :D], Wd[w][:P, :])
                nc.sync.dma_start(W_sb[w][:, D:], Wd[w][P:, :])
            b_sb = {}
            for b in ("b1", "b2"):
                b_sb[b] = const.tile([1, D], F32, name=f"b_{b}")
                nc.sync.dma_start(b_sb[b][:], bd[b][:])
            g1_sb = const.tile([P, NTC], I32)
            nc.sync.dma_start(g1_sb[:], g1_d[:])
            df1_sb = const.tile([P, NTC], F32)
            nc.sync.dma_start(df1_sb[:], df1_d[:])
            g2_sb = const.tile([P, NTC], I32)
            nc.sync.dma_start(g2_sb[:], g2_d[:])
            df2_sb = const.tile([P, NTC], F32)
            nc.sync.dma_start(df2_sb[:], df2_d[:])
            selfi_sb = const.tile([P, BINS_PER_CORE], I32)
            nc.sync.dma_start(selfi_sb[:], selfi_d[:])
            cnt_sb = const.tile([P, BINS_PER_CORE], F32)
            nc.sync.dma_start(cnt_sb[:], cnt_d[:])
            recip = const.tile([P, BINS_PER_CORE], F32)
            nc.vector.tensor_scalar(
                out=recip[:], in0=cnt_sb[:], scalar1=1.0, scalar2=None,
                op0=mybir.AluOpType.max)
            nc.vector.reciprocal(recip[:], recip[:])

            # ---- scratch DRAM
            x1own = dram.tile([NODE_SLOTS, D], F32)
            ag_in = dram.tile([NODE_SLOTS, D], BF16)
            ag_out = dram.tile([NC * NODE_SLOTS, D], BF16, addr_space="Shared")

            def layer(L, gsrc, gidx_sb, dstf_sb, Wl, Wr, bias, self_load, writer):
                for b in range(BINS_PER_CORE):
                    agg_ps = psA.tile([P, D], F32, name=f"agg{L}_{b}", tag="agg")
                    for t in range(T):
                        col = b * T + t
                        g = gp.tile([P, D], BF16, name=f"g{L}_{b}_{t}", tag="g")
                        nc.gpsimd.indirect_dma_start(
                            out=g[:], out_offset=None, in_=gsrc[:],
                            in_offset=bass.IndirectOffsetOnAxis(
                                ap=gidx_sb[:, col:col + 1], axis=0))
                        oh = ohp.tile([P, P], BF16, name=f"oh{L}_{b}_{t}", tag="oh")
                        nc.vector.tensor_tensor(
                            out=oh[:],
                            in0=dstf_sb[:, col:col + 1].to_broadcast([P, P]),
                            in1=iota_sb[:], op=mybir.AluOpType.is_equal)
                        nc.tensor.matmul(agg_ps[:], lhsT=oh[:], rhs=g[:],
                                         start=(t == 0), stop=(t == T - 1))
                    mean_sb = work.tile([P, D], F32, name=f"mean{L}_{b}", tag="mean")
                    nc.vector.tensor_tensor(
                        out=mean_sb[:], in0=agg_ps[:],
                        in1=recip[:, b:b + 1].to_broadcast([P, D]),
                        op=mybir.AluOpType.mult)
                    gs = self_load(b)
                    out_ps = psO.tile([P, D], F32, name=f"out{L}_{b}", tag="out")
                    for c in range(2):
                        tr = psT.tile([P, P], F32, name=f"trm{L}_{b}_{c}", tag="tr")
                        nc.tensor.transpose(tr[:], in_=mean_sb[:, c * P:(c + 1) * P],
                                            identity=ident[:])
                        mT = work.tile([P, P], F32, name=f"mT{L}_{b}_{c}", tag="mT")
                        nc.vector.tensor_copy(mT[:], tr[:])
                        nc.tensor.matmul(out_ps[:], lhsT=mT[:],
                                         rhs=Wl[:, c * D:(c + 1) * D],
                                         start=(c == 0), stop=False)
                    for c in range(2):
                        tr = psT.tile([P, P], F32, name=f"trx{L}_{b}_{c}", tag="tr")
                        nc.tensor.transpose(tr[:], in_=gs[:, c * P:(c + 1) * P],
                                            identity=ident[:])
                        xT = work.tile([P, P], F32, name=f"xT{L}_{b}_{c}", tag="xT")
                        nc.vector.tensor_copy(xT[:], tr[:])
                        nc.tensor.matmul(out_ps[:], lhsT=xT[:],
                                         rhs=Wr[:, c * D:(c + 1) * D],
                                         start=False, stop=False)
                    nc.tensor.matmul(out_ps[:], lhsT=ones_sb[:], rhs=bias[:],
                                     start=False, stop=True)
                    res = outp.tile([P, D], F32, name=f"res{L}_{b}", tag="res")
                    nc.vector.tensor_copy(res[:], out_ps[:])
                    writer(b, res)

            # ---- layer 1
            def self_load1(b):
                gs = selfp.tile([P, D], F32, name=f"gs1_{b}", tag="gs")
                nc.gpsimd.indirect_dma_start(
                    out=gs[:], out_offset=None, in_=emb[:],
                    in_offset=bass.IndirectOffsetOnAxis(
                        ap=selfi_sb[:, b:b + 1], axis=0))
                return gs

            def writer1(b, res):
                nc.sync.dma_start(x1own[b * P:(b + 1) * P, :], res[:])
                bf = outp.tile([P, D], BF16, name=f"resbf_{b}", tag="resbf")
                nc.vector.tensor_copy(bf[:], res[:])
                nc.sync.dma_start(ag_in[b * P:(b + 1) * P, :], bf[:])

            layer(1, emb, g1_sb, df1_sb, W_sb["W1l"], W_sb["W1r"], b_sb["b1"],
                  self_load1, writer1)

            # ---- allgather x1 (bf16)
            nc.gpsimd.collective_compute(
                "AllGather", mybir.AluOpType.bypass,
                replica_groups=[list(range(NC))],
                ins=[ag_in.opt()], outs=[ag_out.opt()])

            # ---- layer 2
            def self_load2(b):
                gs = selfp.tile([P, D], F32, name=f"gs2_{b}", tag="gs")
                nc.sync.dma_start(gs[:], x1own[b * P:(b + 1) * P, :])
                return gs

            def writer2(b, res):
                nc.sync.dma_start(x2_d[b * P:(b + 1) * P, :], res[:])

            layer(2, ag_out, g2_sb, df2_sb, W_sb["W2l"], W_sb["W2r"], b_sb["b2"],
                  self_load2, writer2)

    nc.compile()
    return nc


_PROGRAM_CACHE = {}


def get_program(T):
    if T not in _PROGRAM_CACHE:
        _PROGRAM_CACHE[T] = build_program(T)
    return _PROGRAM_CACHE[T]


def kernel(**inputs):
    in_maps, info = prepare_host(inputs)
    nc = get_program(info["T"])
    last = None
    for _attempt in range(3):
        try:
            res = run_bass_kernel_spmd(nc, in_maps, core_ids=list(range(NC)))
            return assemble([r for r in res.results], info)
        except Exception as e:  # transient device/tunnel flakes: retry
            last = e
    raise last


if __name__ == "__main__":
    rng = np.random.default_rng(0)
    ins = {
        "emb": rng.uniform(0.001, 0.3, (N_EMB, D)).astype(np.float32),
        "W1l": rng.standard_normal((D, D)).astype(np.float32) / 16,
        "b1l": np.zeros(D, np.float32),
        "W1r": rng.standard_normal((D, D)).astype(np.float32) / 16,
        "W2l": rng.standard_normal((D, D)).astype(np.float32) / 16,
        "b2l": np.zeros(D, np.float32),
        "W2r": rng.standard_normal((D, D)).astype(np.float32) / 16,
        "x_idx": rng.integers(0, N_EMB, N_NODES).astype(np.int32),
        "edge_index": rng.integers(0, N_NODES, (2, E_TOTAL)).astype(np.int32),
        "drugNodes": rng.integers(0, N_NODES, 1024).astype(np.int32),
        "seNodes": rng.integers(0, N_NODES, 1024).astype(np.int32),
    }
    out = kernel(**ins)
    print([o.shape for o in out])
